# revision 14
# baseline (speedup 1.0000x reference)
"""MultiHeadPool Trainium2 kernel (bf16 dual-layout, host-normalized).

Per-core computation (batch b of 8, one per NeuronCore):
  X = others[b]          (N=64, T=512, D=128)
  L = X . qT * scale     contraction over d   -> (T, H, N) logits
  W = softmax_n(L)
  ctx = W . X            contraction over n   -> (T, H, D)

Host ships X in bf16 twice, in the two layouts each matmul wants, so the
PE never transposes and nothing is relayed PSUM->SBUF:
  xq[(j n), tp, d'] = X[n, 2tp+j, d]   (d'==D is a ones column)  - mm2 moving
  xt[d, tp, (j n)]  = X[n, 2tp+j, d]                             - mm1 weights

Per t-pair p:
  mm1: L[(j n), h] = xt[:, p, :].T @ qT_scaled      (bf16 LDW + f=7 matmul)
  exp (ACT, f32 PSUM -> bf16 SBUF) into a persistent block-diagonal E tile
      whose zero off-blocks / pad columns are initialized ONCE
  mm2: ctx-pair = E[:, p-blk].T @ xq-chunk  -> (32, 129) at col-group 32*g2;
      col 128 = softmax denominators via the ones column
Unnormalized ctx + denominators leave PSUM via gpsimd casting DMAs
(f32 -> bf16); the host divides and reassembles (t,h,d).
"""

import sys

for p in ("/opt/trn_rl_repo", "/root/.axon_site/_ro/trn_rl_repo"):
    if p not in sys.path:
        sys.path.append(p)

from contextlib import ExitStack

import numpy as np
import ml_dtypes

import concourse.bacc as bacc
import concourse.bass as bass
import concourse.tile as tile
from concourse import mybir
from concourse.bass_utils import run_bass_kernel_spmd

B, N, T, D, H = 8, 64, 512, 128, 7
CH = 16               # t-pairs per DMA chunk
NG = (T // 2) // CH   # 16 chunks per batch
F32 = mybir.dt.float32
BF16 = mybir.dt.bfloat16

_CACHE = {}


def _body(ctx, tc, xq, xt, qt, ob):
    nc = tc.nc
    E2 = 2 * H            # 14
    M2 = 32               # mm2 stationary cols (14 data + 18 pad)
    DE = D + 1            # 129

    singles = ctx.enter_context(tc.tile_pool(name="singles", bufs=1))
    xqp = ctx.enter_context(tc.tile_pool(name="xqp", bufs=6))
    xtp = ctx.enter_context(tc.tile_pool(name="xtp", bufs=6))
    lp = ctx.enter_context(tc.tile_pool(name="lp", bufs=2, space="PSUM"))
    ctxp = ctx.enter_context(tc.tile_pool(name="ctxp", bufs=4, space="PSUM"))
    stg = ctx.enter_context(tc.tile_pool(name="stg", bufs=3))

    qt_sb = singles.tile([D, H], BF16)
    nc.sync.dma_start(out=qt_sb[:], in_=qt[:])

    # persistent double-buffered E tile: exp overwrites only the diagonal
    # j-blocks each chunk; the zero off-blocks and the pad columns are
    # written once (pad-row outputs are dropped by the host).
    ep = singles.tile([128, 2, CH, M2], BF16)
    for s in range(2):
        nc.scalar.activation(
            out=ep[:, s, :, E2:M2],
            in_=qt_sb[:, 0:1].to_broadcast([128, CH, M2 - E2]),
            func=mybir.ActivationFunctionType.Copy, scale=0.0, bias=1e-3,
        )
        nc.scalar.activation(
            out=ep[0:64, s, :, H:E2],
            in_=qt_sb[0:64, 0:1].to_broadcast([64, CH, H]),
            func=mybir.ActivationFunctionType.Copy, scale=0.0, bias=0.0,
        )
        nc.scalar.activation(
            out=ep[64:128, s, :, 0:H],
            in_=qt_sb[0:64, 0:1].to_broadcast([64, CH, H]),
            func=mybir.ActivationFunctionType.Copy, scale=0.0, bias=0.0,
        )

    def load_chunk(g):
        xtc = xtp.tile([128, CH, D], BF16)
        nc.sync.dma_start(out=xtc[:], in_=xt[:, CH * g: CH * (g + 1), :])
        chunk = xqp.tile([128, CH, DE], BF16)
        nc.scalar.dma_start(out=chunk[:], in_=xq[:, CH * g: CH * (g + 1), :])
        return xtc, chunk

    def mm1(xtc):
        lb = lp.tile([128, CH, H], F32)
        for p in range(CH):
            nc.tensor.matmul(
                lb[:, p, :],
                lhsT=xtc[:, p, :],
                rhs=qt_sb[:],
                start=True, stop=True,
            )
        return lb

    # software pipeline: mm1 of chunk g+1 is emitted before exp/mm2 of
    # chunk g so the PE stream never stalls on the ACT exp
    xtc, chunk = load_chunk(0)
    lb = mm1(xtc)
    nxt = None
    for g in range(NG):
        if g + 1 < NG:
            nxt = load_chunk(g + 1)
            lb_next = mm1(nxt[0])

        e_g = ep[:, g % 2]
        nc.scalar.activation(
            out=e_g[0:64, :, 0:H], in_=lb[0:64],
            func=mybir.ActivationFunctionType.Exp,
        )
        nc.scalar.activation(
            out=e_g[64:128, :, H:E2], in_=lb[64:128],
            func=mybir.ActivationFunctionType.Exp,
        )

        # mm2: 8 pairs per PSUM bank; col-groups iterate fastest so the four
        # 32-col PE tiles run concurrently
        for pair2 in range(CH // 16):
            st = stg.tile([128, 2, 2, DE], BF16)
            for half2 in range(2):
                half = pair2 * 2 + half2
                ctxb = ctxp.tile([128, 2, DE], F32)
                for k in range(2):
                    for g2 in range(4):
                        c = half * 8 + 2 * g2 + k
                        nc.tensor.matmul(
                            ctxb[32 * g2: 32 * (g2 + 1), k, :],
                            lhsT=e_g[:, c, :],
                            rhs=chunk[:, c, :],
                            start=True, stop=True,
                            tile_position=(0, 32 * g2),
                        )
                # unnormalized ctx + denominators, cast f32 -> bf16 on the
                # copy out of PSUM; host divides
                nc.vector.tensor_copy(st[:, half2], ctxb[:])
            ib2 = g * (CH // 16) + pair2
            nc.gpsimd.dma_start(out=ob[ib2], in_=st[:])

        if g + 1 < NG:
            xtc, chunk = nxt
            lb = lb_next


def _build():
    nc = bacc.Bacc("TRN2", target_bir_lowering=False, debug=False)
    xq = nc.dram_tensor("xq", [128, T // 2, D + 1], BF16, kind="ExternalInput")
    xt = nc.dram_tensor("xt", [128, T // 2, D], BF16, kind="ExternalInput")
    qt = nc.dram_tensor("qt", [D, H], BF16, kind="ExternalInput")
    # raw bank layout: (ib2, 128 rows = [g2 x (7j+h | pad)], half2, k, d');
    # d'==D holds the softmax denominator; host divides + reassembles
    ob = nc.dram_tensor("ob", [T // 32, 128, 2, 2, D + 1], BF16,
                        kind="ExternalOutput")
    with tile.TileContext(nc) as tc:
        with ExitStack() as ctx:
            _body(ctx, tc, xq[:], xt[:], qt[:], ob[:])
    nc.compile()
    return nc


def get_nc():
    if "nc" not in _CACHE:
        _CACHE["nc"] = _build()
    return _CACHE["nc"]


def prep_inputs(others_b):
    """others[b] (N,T,D) f32 -> (xq, xt) bf16 layouts."""
    v = others_b.reshape(N, T // 2, 2, D).astype(ml_dtypes.bfloat16)
    xq = np.empty((128, T // 2, D + 1), dtype=ml_dtypes.bfloat16)
    xq[:, :, D] = 1.0
    xq[0:64, :, 0:D] = v[:, :, 0, :]               # j=0 rows 0..63  (n)
    xq[64:128, :, 0:D] = v[:, :, 1, :]             # j=1 rows 64..127
    # xt[d, tp, j*64+n] = X[n, 2tp+j, d]
    xt = np.ascontiguousarray(
        v.transpose(3, 1, 2, 0).reshape(D, T // 2, 128)
    )
    return xq, xt


def kernel(ego=None, others=None, queries=None, _trace=False, **_unused):
    others = np.asarray(others, dtype=np.float32)
    queries = np.asarray(queries, dtype=np.float32)
    scale = float(queries.shape[-1]) ** -0.5
    qt_scaled = np.ascontiguousarray(queries.T * scale).astype(ml_dtypes.bfloat16)

    nc = get_nc()
    in_maps = []
    for b in range(B):
        xq, xt = prep_inputs(others[b])
        in_maps.append({"xq": xq, "xt": xt, "qt": qt_scaled})
    res = run_bass_kernel_spmd(nc, in_maps, core_ids=list(range(B)), trace=_trace)
    _CACHE["last_results"] = res
    out = np.empty((B, T, H, D), dtype=np.float32)
    for b in range(B):
        out[b] = unpack_output(res.results[b]["ob"])
    return out


def unpack_output(ob_raw):
    """(T/32, 128, 2, 2, D+1) bank layout -> (T, H, D);
    t = 16*(2 ib2 + half2) + 4 g2 + 2k + j. Column D is the softmax
    denominator; rows 14..31 of each 32-row strip are pad.
    """
    s = np.asarray(ob_raw, dtype=np.float32)
    s = s.transpose(0, 2, 1, 3, 4).reshape(T // 16, 4, 32, 2, D + 1)
    s = s[:, :, : 2 * H]
    ctx = s[..., :D] / s[..., D:]
    ctx = ctx.reshape(T // 16, 4, 2, H, 2, D)      # ib, g2, j, h, k, d
    return np.ascontiguousarray(
        ctx.transpose(0, 1, 4, 2, 3, 5).reshape(T, H, D)
    )
